# revision 24
# baseline (speedup 1.0000x reference)
"""MLA (multi-head latent attention) Bass kernel for 8 trn2 NeuronCores.

Sharding: core = b*4 + g  (b in {0,1} batches, g in {0..3} head-groups of 4 heads).
Each core: projections from xT (bf16 matmuls), flash-style causal attention with
k-major scores (S^T) so exp'd probs feed PV directly, LOBO softmax
attn = exp(s) / (sum_k exp(s) + C*exp(max_k s)), row-parallel out-proj partial.
Host sums the 4 partials per batch.

v2 layout notes:
  - PSUM tags: S = [128,1536] (3 banks) x2, Y = [128,512] x2  -> 8 banks total.
  - Projections pack (kv0,kv1,kr) and (q01,q23,qr) each into one S tile.
  - Attention: per (h,qg) score chunks land 3-per-S-tile; one wide exp per
    off-diagonal run, granular exps + DVE-memset + gpsimd triangle-mask on the
    4 diagonal chunks.  Per-query running max kept in a [128,T] bf16 comb tile
    (DVE tensor_max); partition-max via ONE gpsimd tensor_reduce(axis=C).
  - Denominator D rides a ones-column in V through the PV matmul (row 64).
  - y normalization: r broadcast to 64-row blocks via a tiny f32r matmul with a
    0/1 selector lhsT; yT2 multiplied in place.
"""

import math
import os

import ml_dtypes
import numpy as np

BF16NP = ml_dtypes.bfloat16

import concourse.bass as bass
import concourse.bass_isa as bass_isa
import concourse.mybir as mybir
import concourse.tile as _tile_mod
from concourse.tile import TileContext
from concourse.vector_clock import ScopedClock, VectorClock
import bass_rust as _bass_rust
from concourse.bass_utils import run_bass_kernel_spmd

_N_PROCS = _bass_rust.N_PROCS


def _split_drain_and_barrier(self, tick_clock, wait_clock):
    """Replacement for TileContext._drain_and_barrier: the stock version puts
    the whole global vector clock (up to 27 sem waits) on one Drain, which this
    walrus rejects ("Too many sync wait commands").  Emit one Drain per
    outstanding processor instead."""
    gc = tick_clock.global_clock
    procs = [p for p in range(_N_PROCS) if gc[p] > 0]
    for p in procs:
        vc = VectorClock([gc[q] if q == p else 0 for q in range(_N_PROCS)])
        d = self.nc.sync.drain()
        wait_clock.add_sem_waits(d.ins, ScopedClock({None: vc}))
    self.nc.all_engine_barrier()
    popped = self.nc._tile_sem_poison_stack.pop()
    assert popped is self._sem_poison
    self.nc.clear_and_free_semaphores(list(self.sems.allocated().values()))
    self.nc.all_engine_barrier()


_tile_mod.TileContext._drain_and_barrier = _split_drain_and_barrier

# ---------------------------------------------------------------------------
# This walrus build enforces small per-instruction sync-wait budgets
# ("Too many sync wait commands").  Post-process the BIR JSON: any
# instruction carrying more than its budget of waits gets the excess
# hoisted onto same-engine Drain carriers inserted immediately before it
# (same program point on the engine's sequential stream -> semantics
# unchanged).
# ---------------------------------------------------------------------------
_orig_to_json_bytes = bass.Bass.to_json_bytes
_WAIT_LIMITS = {"Drain": 1, "DMACopy": 1}
_DEF_WAIT_LIMIT = 1


def _to_json_split_waits(self, *a, **kw):
    import json as _json
    data = _json.loads(_orig_to_json_bytes(self, *a, **kw))
    nid = 0
    for f in data.get("functions", []):
        for bb in f.get("blocks", []):
            out = []
            for inst in bb.get("instructions", []):
                si = inst.get("sync_info")
                if isinstance(si, dict):
                    w = si.get("on_wait")
                    if isinstance(w, list):
                        k = _WAIT_LIMITS.get(inst.get("opcode"), _DEF_WAIT_LIMIT)
                        if len(w) > k:
                            extra, keep = w[:-k], w[-k:]
                            for wt in extra:
                                out.append({
                                    "debug": inst.get("debug"),
                                    "engine": inst["engine"],
                                    "ins": [], "outs": [],
                                    "name": f"wsplit-{nid}",
                                    "opcode": "Drain",
                                    "sync_info": {"on_update": [],
                                                  "on_wait": [wt]},
                                })
                                nid += 1
                            si["on_wait"] = keep
                out.append(inst)
            bb["instructions"] = out
    return _json.dumps(data).encode()


bass.Bass.to_json_bytes = _to_json_split_waits

B, T, E = 2, 2048, 1024
H, DH = 16, 64
DKV = 256
DR = 32
HL = 4              # heads per core
NG = 4              # head groups
SCALE = 1.0 / math.sqrt(DH + DR)
TG = 512            # query-group width
KC = 128            # key-chunk width
NTG = T // TG       # 4
NKC = T // KC       # 16
EC = E // 128       # 8  e-chunks
CC = DKV // 128     # 2  latent chunks

F32 = mybir.dt.float32
F32R = mybir.dt.float32r
BF16 = mybir.dt.bfloat16
AF = mybir.ActivationFunctionType
ALU = mybir.AluOpType
AX = mybir.AxisListType

SWAP16 = list(range(16, 32)) + list(range(0, 16))

_CACHE = {}


def _r(ap):
    return ap.bitcast(F32R)


def _build_program():
    nc = bass.Bass()

    xT = nc.declare_dram_parameter("xT", [E, T], BF16, isOutput=False)
    wq = nc.declare_dram_parameter("wq", [E, HL * DH], BF16, isOutput=False)
    wqr = nc.declare_dram_parameter("wqr", [E, HL * DR], BF16, isOutput=False)
    wkr = nc.declare_dram_parameter("wkr", [E, DR], BF16, isOutput=False)
    wkvd = nc.declare_dram_parameter("wkvd", [E, DKV], BF16, isOutput=False)
    wku = nc.declare_dram_parameter("wku", [DKV, HL * DH], BF16, isOutput=False)
    wvu = nc.declare_dram_parameter("wvu", [DKV, HL * DH], BF16, isOutput=False)
    wo = nc.declare_dram_parameter("wo", [HL * DH, E], BF16, isOutput=False)
    cosq = nc.declare_dram_parameter("cosq", [HL * DR, T], BF16, isOutput=False)
    sinq = nc.declare_dram_parameter("sinq", [HL * DR, T], BF16, isOutput=False)
    tri = nc.declare_dram_parameter("tri", [128, 128], BF16, isOutput=False)
    sel = nc.declare_dram_parameter("sel", [66, 128], BF16, isOutput=False)
    lobo = nc.declare_dram_parameter("lobo", [66, 1], F32, isOutput=False)
    out = nc.declare_dram_parameter("out", [T, E], BF16, isOutput=True)

    with TileContext(nc) as tc:
        from contextlib import ExitStack

        with ExitStack() as ctx:
            singles = ctx.enter_context(tc.tile_pool(name="singles", bufs=1))
            pool = ctx.enter_context(tc.tile_pool(name="pool", bufs=2))
            psp = ctx.enter_context(tc.tile_pool(name="psp", bufs=1, space="PSUM"))

            # ---------------- weights (bf16 in SBUF) ----------------
            # Each issuing engine owns one ~22.5 GB/s DMA ring, so big
            # loads are chunked round-robin across sync/scalar/vector/gpsimd
            # in consumption order (tg0's x + kv/q weights first).
            QS = [nc.sync, nc.scalar, nc.gpsimd]
            qi = [0]

            def ld(out_ap, in_ap):
                QS[qi[0] % 3].dma_start(out=out_ap, in_=in_ap)
                qi[0] += 1

            xt_sb = singles.tile([128, EC, T], BF16)
            xT_r = xT.rearrange("(c p) t -> p c t", p=128)
            wkvd_sb = singles.tile([128, EC, DKV], BF16)
            wkvd_r = wkvd.rearrange("(c p) f -> p c f", p=128)
            wq_sb = singles.tile([128, EC, HL * DH], BF16)
            wq_r = wq.rearrange("(c p) f -> p c f", p=128)
            wqr_sb = singles.tile([128, EC, HL * DR], BF16)
            wkr_sb = singles.tile([128, EC, DR], BF16)
            for e0 in range(0, EC, 2):
                ld(wkvd_sb[:, e0:e0 + 2, :], wkvd_r[:, e0:e0 + 2, :])
                ld(xt_sb[:, e0:e0 + 2, 0:TG], xT_r[:, e0:e0 + 2, 0:TG])
            nc.sync.dma_start(
                out=wkr_sb, in_=wkr.rearrange("(c p) f -> p c f", p=128))
            for e0 in range(0, EC, 2):
                ld(wq_sb[:, e0:e0 + 2, :], wq_r[:, e0:e0 + 2, :])
            ld(wqr_sb, wqr.rearrange("(c p) f -> p c f", p=128))
            wku_sb = singles.tile([128, CC, HL * DH], BF16)
            ld(wku_sb, wku.rearrange("(c p) f -> p c f", p=128))
            wvu_sb = singles.tile([128, CC, HL * DH], BF16)
            ld(wvu_sb, wvu.rearrange("(c p) f -> p c f", p=128))
            cosq_sb = singles.tile([128, T], BF16)
            sinq_sb = singles.tile([128, T], BF16)
            ld(cosq_sb[:, 0:TG], cosq[:, 0:TG])
            ld(sinq_sb[:, 0:TG], sinq[:, 0:TG])
            tri_sb = singles.tile([128, 128], BF16)
            nc.sync.dma_start(out=tri_sb, in_=tri[:, :])
            sel_sb = singles.tile([66, 128], BF16)
            nc.sync.dma_start(out=sel_sb, in_=sel[:, :])
            lobo_sb = singles.tile([66, 1], F32)
            nc.sync.dma_start(out=lobo_sb, in_=lobo[:, :])
            c_sb = singles.tile([66, 1], F32)
            nc.scalar.activation(c_sb, lobo_sb, AF.Exp)
            for t0 in range(TG, T, TG):
                tsl = slice(t0, t0 + TG)
                ld(cosq_sb[:, tsl], cosq[:, tsl])
                ld(sinq_sb[:, tsl], sinq[:, tsl])
            wo_sb = singles.tile([128, 2, E], BF16)
            ld(wo_sb[:, :, 0:TG], wo.rearrange("(c p) e -> p c e", p=128)[:, :, 0:TG])
            ld(wo_sb[:, :, TG:2 * TG],
               wo.rearrange("(c p) e -> p c e", p=128)[:, :, TG:2 * TG])

            # ---------------- persistent activation tiles ----------------
            latT_sb = singles.tile([128, CC, T], BF16)
            # head h lives at slot SLOT[h] so staging DMAs can pair heads
            SLOT = [0, 2, 1, 3]
            qTall = singles.tile([96, HL, T], BF16)
            kTall = singles.tile([96, HL, T], BF16)
            v_sb = singles.tile([128, NKC, HL, DH + 1], BF16)
            nc.vector.memset(v_sb[:, :, :, DH:DH + 1], 1.0)
            yT2 = singles.tile([128, 2, T], BF16)
            # head h row lives at partition 64*(h//2) + h%2
            dsum_sb = singles.tile([66, T], F32)
            emax_sb = singles.tile([66, T], F32)
            emst_sb = singles.tile([66, T], F32)
            nc.vector.memset(dsum_sb, 1.0)
            nc.vector.memset(emax_sb, 1.0)
            nc.vector.memset(emst_sb, 1.0)

            # =================== projections, per tg ===================
            for tg in range(NTG):
                ts = slice(tg * TG, (tg + 1) * TG)
                if tg > 0:
                    ld(xt_sb[:, 0:4, ts], xT_r[:, 0:4, ts])
                    ld(xt_sb[:, 4:8, ts], xT_r[:, 4:8, ts])
                xts = [xt_sb[:, ec, ts] for ec in range(EC)]

                # --- latent (kv) halves into S tile A ---
                ska = psp.tile([128, 2 * TG], F32, name="ska", tag="S", bufs=2)
                for ec in range(EC):
                    nc.tensor.matmul(
                        ska[:, 0:TG], wkvd_sb[:, ec, 0:128], xts[ec],
                        start=(ec == 0), stop=(ec == EC - 1))
                for ec in range(EC):
                    nc.tensor.matmul(
                        ska[:, TG:2 * TG], wkvd_sb[:, ec, 128:256], xts[ec],
                        start=(ec == 0), stop=(ec == EC - 1))
                nc.scalar.copy(
                    latT_sb[:, :, ts],
                    ska.rearrange("p (c t) -> p c t", c=2))

                # --- k_rope + q_rope into S tile B ---
                skb = psp.tile([128, 2 * TG], F32, name="skb", tag="S", bufs=2)
                for ec in range(EC):
                    nc.tensor.matmul(
                        skb[0:DR, 0:TG], wkr_sb[:, ec, :], xts[ec],
                        start=(ec == 0), stop=(ec == EC - 1))
                for ec in range(EC):
                    nc.tensor.matmul(
                        skb[:, TG:2 * TG], wqr_sb[:, ec, :], xts[ec],
                        start=(ec == 0), stop=(ec == EC - 1))
                kr_pre = pool.tile([DR, TG], BF16, name="krp", tag="krp", bufs=2)
                nc.scalar.copy(kr_pre, skb[0:DR, 0:TG])
                rp_pre = pool.tile([128, TG], BF16, name="rpp", tag="rpp", bufs=2)
                nc.scalar.copy(rp_pre, skb[:, TG:2 * TG])

                # k_rope rotate-half + tables
                kr_sw = pool.tile([DR, TG], BF16, name="krs", tag="krs", bufs=2)
                nc.vector.stream_shuffle(
                    kr_sw, kr_pre, mask=SWAP16)
                kr_m = pool.tile([DR, TG], BF16, name="krm", tag="krm", bufs=2)
                nc.vector.tensor_mul(kr_m, kr_pre, cosq_sb[0:DR, ts])
                nc.vector.tensor_mul(kr_sw, kr_sw, sinq_sb[0:DR, ts])
                nc.vector.tensor_add(kr_m, kr_m, kr_sw)
                for h in range(HL):
                    nc.sync.dma_start(
                        out=kTall[DH:96, SLOT[h], ts], in_=kr_m)

                # q_rope rotate-half + tables
                rp_sw = pool.tile([128, TG], BF16, name="rps", tag="rps", bufs=2)
                nc.vector.stream_shuffle(rp_sw, rp_pre, mask=SWAP16)
                rp_m = pool.tile([128, TG], BF16, name="rpm", tag="rpm", bufs=2)
                nc.vector.tensor_mul(rp_m, rp_pre, cosq_sb[:, ts])
                nc.vector.tensor_mul(rp_sw, rp_sw, sinq_sb[:, ts])
                nc.vector.tensor_add(rp_m, rp_m, rp_sw)
                for h in range(HL):
                    nc.sync.dma_start(
                        out=qTall[DH:96, SLOT[h], ts],
                        in_=rp_m[h * DR:(h + 1) * DR, :])

                # --- q halves into S tile C ---
                skc2 = psp.tile([128, 2 * TG], F32, name="skc2", tag="S", bufs=2)
                for ec in range(EC):
                    nc.tensor.matmul(
                        skc2[:, 0:TG], wq_sb[:, ec, 0:128], xts[ec],
                        start=(ec == 0), stop=(ec == EC - 1))
                for ec in range(EC):
                    nc.tensor.matmul(
                        skc2[:, TG:2 * TG], wq_sb[:, ec, 128:256], xts[ec],
                        start=(ec == 0), stop=(ec == EC - 1))
                stq = pool.tile([128, 2 * TG], BF16, name="stq", tag="stq", bufs=2)
                nc.scalar.copy(stq, skc2)
                nc.sync.dma_start(
                    out=qTall[0:DH, 0:2, ts],
                    in_=stq[0:DH, :].rearrange("p (k t) -> p k t", k=2))
                nc.sync.dma_start(
                    out=qTall[0:DH, 2:4, ts],
                    in_=stq[DH:128, :].rearrange("p (k t) -> p k t", k=2))

                # --- k_c from latent into S tile D ---
                skd = psp.tile([128, 2 * TG], F32, name="skd", tag="S", bufs=2)
                for cc in range(CC):
                    nc.tensor.matmul(
                        skd[:, 0:TG], wku_sb[:, cc, 0:128], latT_sb[:, cc, ts],
                        start=(cc == 0), stop=(cc == CC - 1))
                for cc in range(CC):
                    nc.tensor.matmul(
                        skd[:, TG:2 * TG], wku_sb[:, cc, 128:256],
                        latT_sb[:, cc, ts],
                        start=(cc == 0), stop=(cc == CC - 1))
                stk = pool.tile([128, 2 * TG], BF16, name="stk", tag="stq", bufs=2)
                nc.scalar.copy(stk, skd)
                nc.sync.dma_start(
                    out=kTall[0:DH, 0:2, ts],
                    in_=stk[0:DH, :].rearrange("p (k t) -> p k t", k=2))
                nc.sync.dma_start(
                    out=kTall[0:DH, 2:4, ts],
                    in_=stk[DH:128, :].rearrange("p (k t) -> p k t", k=2))

                # --- V (natural layout) for this tg's 4 key chunks ---
                for half in range(2):
                    kc0 = 4 * tg + 2 * half
                    vps = psp.tile([128, TG], F32, name="vps", tag="Y", bufs=2)
                    for cc in range(CC):
                        nc.tensor.matmul(
                            vps[:, 0:256],
                            latT_sb[:, cc, kc0 * KC:(kc0 + 1) * KC],
                            wvu_sb[:, cc, :],
                            start=(cc == 0), stop=(cc == CC - 1))
                    for cc in range(CC):
                        nc.tensor.matmul(
                            vps[:, 256:512],
                            latT_sb[:, cc, (kc0 + 1) * KC:(kc0 + 2) * KC],
                            wvu_sb[:, cc, :],
                            start=(cc == 0), stop=(cc == CC - 1))
                    nc.scalar.copy(
                        v_sb[:, kc0:kc0 + 2, :, 0:DH],
                        vps.rearrange("p (k h d) -> p k h d", k=2, h=HL))

            # =================== attention ===================
            # two heads of a pair run as interleaved independent pipelines
            comb2 = singles.tile([128, 2, T], BF16)
            r_bf = singles.tile([66, T], BF16)
            emst_r = emst_sb.rearrange("p (i b) -> p i b", b=64)
            for hp2 in range(2):
                heads = (2 * hp2, 2 * hp2 + 1)
                for qg in range(NTG):
                    qs = slice(qg * TG, (qg + 1) * TG)
                    nkc = 4 * (qg + 1)
                    groups = [list(range(g0, min(g0 + 2, nkc)))
                              for g0 in range(0, nkc, 2)]
                    yp = {}
                    for h in heads:
                        yp[h] = psp.tile(
                            [128, TG], F32, name=f"yps{h % 2}", tag="Y",
                            bufs=2)
                    pts = {}
                    prev_pv = None

                    def joff(c):
                        # valid columns of chunk c start at j*KC (diag)
                        j = c - (nkc - 4)
                        return j * KC if j > 0 else 0

                    def emit_pv(gi):
                        for h in heads:
                            ptt, chunks = pts[(gi, h)]
                            for li, c in enumerate(chunks):
                                o = joff(c)
                                nc.tensor.matmul(
                                    yp[h][0:DH + 1, o:TG], v_sb[:, c, h, :],
                                    ptt[:, li, o:TG],
                                    start=(c == 0), stop=(c == nkc - 1),
                                    skip_group_check=True)

                    for gi, chunks in enumerate(groups):
                        sp = {}
                        for h in heads:
                            sp[h] = psp.tile(
                                [128, 2 * TG], F32, name=f"sps{h % 2}",
                                tag="S", bufs=2)
                            for li, c in enumerate(chunks):
                                o = joff(c)
                                nc.tensor.matmul(
                                    sp[h][:, li * TG + o:(li + 1) * TG],
                                    kTall[:, SLOT[h], c * KC:(c + 1) * KC],
                                    qTall[:, SLOT[h],
                                          qg * TG + o:(qg + 1) * TG])
                        if prev_pv is not None:
                            emit_pv(prev_pv)
                        for h in heads:
                            ptt = pool.tile(
                                [128, 2, TG], BF16, name=f"pt{h % 2}",
                                tag="pt", bufs=6)
                            pts[(gi, h)] = (ptt, chunks)
                            li = 0
                            while li < len(chunks):
                                c = chunks[li]
                                j = c - (nkc - 4)
                                if j < 0:
                                    l1 = li
                                    while (l1 < len(chunks)
                                           and chunks[l1] - (nkc - 4) < 0):
                                        l1 += 1
                                    nc.scalar.activation(
                                        ptt[:, li:l1, :],
                                        sp[h][:, li * TG:l1 * TG].rearrange(
                                            "p (k t) -> p k t", k=l1 - li),
                                        AF.Exp, scale=SCALE)
                                    li = l1
                                else:
                                    o = j * KC
                                    nc.scalar.activation(
                                        ptt[:, li, o:TG],
                                        sp[h][:, li * TG + o:(li + 1) * TG],
                                        AF.Exp, scale=SCALE)
                                    nc.gpsimd.tensor_mul(
                                        ptt[:, li, o:o + KC],
                                        ptt[:, li, o:o + KC], tri_sb)
                                    li += 1
                        for h in heads:
                            ptt, _ = pts[(gi, h)]
                            cslot = comb2[:, h % 2, :]
                            for li, c in enumerate(chunks):
                                o = joff(c)
                                if c == 0:
                                    nc.vector.tensor_copy(
                                        cslot[:, qs], ptt[:, li, :])
                                else:
                                    nc.vector.tensor_max(
                                        cslot[:, qg * TG + o:(qg + 1) * TG],
                                        cslot[:, qg * TG + o:(qg + 1) * TG],
                                        ptt[:, li, o:TG])
                        prev_pv = gi
                    emit_pv(prev_pv)

                    for h in heads:
                        # stage y + D, scatter to yT2 / dsum
                        hp = 64 * (h // 2) + h % 2
                        st65 = pool.tile(
                            [DH + 1, TG], F32, name="st65", tag="st65",
                            bufs=4)
                        nc.vector.tensor_copy(st65, yp[h][0:DH + 1, :])
                        nc.gpsimd.dma_start(
                            out=yT2[(h % 2) * DH:(h % 2 + 1) * DH,
                                    h // 2, qs],
                            in_=st65[0:DH, :])
                        nc.sync.dma_start(
                            out=dsum_sb[hp:hp + 1, qs],
                            in_=st65[DH:DH + 1, :])

                        # partition-max of comb[:, qs] for this head
                        combT = pool.tile(
                            [128, TG], BF16, name="combT", tag="combT",
                            bufs=2)
                        nc.vector.transpose(combT, comb2[:, h % 2, qs])
                        red = pool.tile(
                            [128, TG // 32], F32, name="red", tag="red",
                            bufs=2)
                        nc.vector.reduce_max(
                            red, combT.rearrange("p (b j) -> p b j", j=32),
                            axis=AX.X)
                        stkt = pool.tile(
                            [32, 4, TG // 32], F32, name="stkt", tag="stkt",
                            bufs=2)
                        for a in range(4):
                            nc.sync.dma_start(
                                out=stkt[:, a, :],
                                in_=red[a * 32:(a + 1) * 32, :])
                        emf = pool.tile(
                            [32, TG // 32], F32, name="emf", tag="emf",
                            bufs=2)
                        nc.vector.reduce_max(
                            emf, stkt.rearrange("p a b -> p b a"), axis=AX.X)
                        nc.sync.dma_start(
                            out=emst_sb[hp:hp + 1, qs].rearrange(
                                "p (i b) -> p i b", i=32),
                            in_=emf)

                    # this pair's denominator chain for columns qs
                    rows = slice(64 * hp2, 64 * hp2 + 2)
                    nc.vector.tensor_copy(
                        emax_sb[rows, qs].rearrange("p (b i) -> p i b", i=32),
                        emst_sb[rows, qs].rearrange("p (i b) -> p i b", i=32))
                    nc.vector.scalar_tensor_tensor(
                        out=dsum_sb[rows, qs], in0=emax_sb[rows, qs],
                        scalar=c_sb[rows, :], in1=dsum_sb[rows, qs],
                        op0=ALU.mult, op1=ALU.add)
                    lnd = pool.tile(
                        [2, TG], F32, name="lnd", tag="lnd", bufs=2)
                    nc.scalar.activation(lnd, dsum_sb[rows, qs], AF.Ln)
                    nc.scalar.activation(
                        r_bf[rows, qs], lnd, AF.Exp, scale=-1.0)

                    # normalize this pair's half of yT2 for columns qs
                    bcps = psp.tile([128, TG], F32, name="bc", tag="Y",
                                    bufs=2)
                    nc.tensor.matmul(
                        bcps, sel_sb[64 * hp2:64 * hp2 + 2, :],
                        r_bf[rows, qs])
                    nc.vector.tensor_mul(
                        yT2[:, hp2, qs], yT2[:, hp2, qs], bcps)

                    if hp2 == 1:
                        # both pairs normalized for qs -> project out now
                        for tt in range(4 * qg, 4 * (qg + 1)):
                            for eg in range(2):
                                ops = psp.tile(
                                    [128, TG], F32, name="ops", tag="O",
                                    bufs=2)
                                for fc in range(2):
                                    nc.tensor.matmul(
                                        ops,
                                        yT2[:, fc, tt * KC:(tt + 1) * KC],
                                        wo_sb[:, fc,
                                              eg * TG:(eg + 1) * TG],
                                        start=(fc == 0), stop=(fc == 1))
                                ost = pool.tile(
                                    [128, TG], BF16, name="ost", tag="ost",
                                    bufs=3)
                                if (tt * 2 + eg) % 2 == 0:
                                    nc.scalar.copy(ost, ops)
                                else:
                                    nc.vector.tensor_copy(ost, ops)
                                nc.sync.dma_start(
                                    out=out[tt * KC:(tt + 1) * KC,
                                            eg * TG:(eg + 1) * TG],
                                    in_=ost)

    return nc


def _tri():
    x = np.arange(128)[:, None]
    y = np.arange(128)[None, :]
    return (x <= y).astype(np.float32)  # keep iff key row <= query col


def _sel():
    s = np.zeros((66, 128), dtype=np.float32)
    for base in (0, 64):
        s[base + 0, 0:64] = 1.0
        s[base + 1, 64:128] = 1.0
    return s


def _lobo66(lg):
    v = np.zeros((66, 1), dtype=np.float32)
    for h in range(HL):
        v[64 * (h // 2) + h % 2, 0] = lg[h]
    return v


def _rope_tables():
    half = DR // 2
    inv = 1.0 / (10000.0 ** (np.arange(half, dtype=np.float64) / half))
    ang = np.arange(T, dtype=np.float64)[:, None] * inv[None, :]  # (T, half)
    cos = np.cos(ang).T  # (half, T)
    sin = np.sin(ang).T
    cosk = np.concatenate([cos, cos], axis=0)                 # (32, T)
    sink = np.concatenate([-sin, sin], axis=0)
    cosq = np.tile(cosk, (HL, 1)).astype(np.float32)          # (128, T)
    sinq = np.tile(sink, (HL, 1)).astype(np.float32)
    return cosq, sinq


def kernel(x, Wq, Wqr, Wkr, Wkvd, Wku, Wvu, Wo, lobo_log):
    x = np.asarray(x, dtype=np.float32)
    Wq = np.asarray(Wq, dtype=np.float32)
    Wqr = np.asarray(Wqr, dtype=np.float32)
    Wkr = np.asarray(Wkr, dtype=np.float32)
    Wkvd = np.asarray(Wkvd, dtype=np.float32)
    Wku = np.asarray(Wku, dtype=np.float32)
    Wvu = np.asarray(Wvu, dtype=np.float32)
    Wo = np.asarray(Wo, dtype=np.float32)
    lobo_log = np.asarray(lobo_log, dtype=np.float32)

    if "nc" not in _CACHE:
        _CACHE["nc"] = _build_program()
    nc = _CACHE["nc"]

    cosq, sinq = _rope_tables()
    bf = lambda a: np.ascontiguousarray(a).astype(BF16NP)
    xTb = [bf(x[b].T) for b in range(B)]
    wkr_b, wkvd_b = bf(Wkr), bf(Wkvd)
    cosq_b, sinq_b = bf(cosq), bf(sinq)
    tri_b, sel_b = bf(_tri()), bf(_sel())
    in_maps = []
    for core in range(8):
        b, g = core // NG, core % NG
        hs = slice(g * HL * DH, (g + 1) * HL * DH)
        rs = slice(g * HL * DR, (g + 1) * HL * DR)
        in_maps.append({
            "xT": xTb[b],
            "wq": bf(Wq[:, hs]),
            "wqr": bf(Wqr[:, rs]),
            "wkr": wkr_b,
            "wkvd": wkvd_b,
            "wku": bf(Wku[:, hs]),
            "wvu": bf(Wvu[:, hs]),
            "wo": bf(Wo[hs, :]),
            "cosq": cosq_b, "sinq": sinq_b,
            "tri": tri_b, "sel": sel_b,
            "lobo": _lobo66(lobo_log[g * HL:(g + 1) * HL]),
        })

    trace = bool(os.environ.get("BASS_TRACE_KERNEL"))
    bkr = run_bass_kernel_spmd(
        nc, in_maps, core_ids=list(range(8)), trace=trace)
    if trace:
        print(f"HW exec time: {bkr.exec_time_ns} ns")
        if bkr.instructions_and_trace is not None:
            print("trace:", bkr.instructions_and_trace[1])
        _CACHE["last_result"] = bkr
    res = bkr.results
    out = np.zeros((B, T, E), dtype=np.float32)
    for core in range(8):
        out[core // NG] += np.asarray(res[core]["out"], dtype=np.float32)
    return out


# revision 27
# speedup vs baseline: 1.1378x; 1.1378x over previous
"""MLA (multi-head latent attention) Bass kernel for 8 trn2 NeuronCores.

Sharding: core = b*4 + g  (b in {0,1} batches, g in {0..3} head-groups of 4 heads).
Each core: projections from xT (bf16 matmuls), flash-style causal attention with
k-major scores (S^T) so exp'd probs feed PV directly, LOBO softmax
attn = exp(s) / (sum_k exp(s) + C*exp(max_k s)), row-parallel out-proj partial.
Host sums the 4 partials per batch.

v2 layout notes:
  - PSUM tags: S = [128,1536] (3 banks) x2, Y = [128,512] x2  -> 8 banks total.
  - Projections pack (kv0,kv1,kr) and (q01,q23,qr) each into one S tile.
  - Attention: per (h,qg) score chunks land 3-per-S-tile; one wide exp per
    off-diagonal run, granular exps + DVE-memset + gpsimd triangle-mask on the
    4 diagonal chunks.  Per-query running max kept in a [128,T] bf16 comb tile
    (DVE tensor_max); partition-max via ONE gpsimd tensor_reduce(axis=C).
  - Denominator D rides a ones-column in V through the PV matmul (row 64).
  - y normalization: r broadcast to 64-row blocks via a tiny f32r matmul with a
    0/1 selector lhsT; yT2 multiplied in place.
"""

import math
import os

import ml_dtypes
import numpy as np

BF16NP = ml_dtypes.bfloat16

import concourse.bass as bass
import concourse.bass_isa as bass_isa
import concourse.mybir as mybir
import concourse.tile as _tile_mod
from concourse.tile import TileContext
from concourse.vector_clock import ScopedClock, VectorClock
import bass_rust as _bass_rust
from concourse.bass_utils import run_bass_kernel_spmd

_N_PROCS = _bass_rust.N_PROCS


def _split_drain_and_barrier(self, tick_clock, wait_clock):
    """Replacement for TileContext._drain_and_barrier: the stock version puts
    the whole global vector clock (up to 27 sem waits) on one Drain, which this
    walrus rejects ("Too many sync wait commands").  Emit one Drain per
    outstanding processor instead."""
    gc = tick_clock.global_clock
    procs = [p for p in range(_N_PROCS) if gc[p] > 0]
    for p in procs:
        vc = VectorClock([gc[q] if q == p else 0 for q in range(_N_PROCS)])
        d = self.nc.sync.drain()
        wait_clock.add_sem_waits(d.ins, ScopedClock({None: vc}))
    self.nc.all_engine_barrier()
    popped = self.nc._tile_sem_poison_stack.pop()
    assert popped is self._sem_poison
    self.nc.clear_and_free_semaphores(list(self.sems.allocated().values()))
    self.nc.all_engine_barrier()


_tile_mod.TileContext._drain_and_barrier = _split_drain_and_barrier

# ---------------------------------------------------------------------------
# This walrus build enforces small per-instruction sync-wait budgets
# ("Too many sync wait commands").  Post-process the BIR JSON: any
# instruction carrying more than its budget of waits gets the excess
# hoisted onto same-engine Drain carriers inserted immediately before it
# (same program point on the engine's sequential stream -> semantics
# unchanged).
# ---------------------------------------------------------------------------
_orig_to_json_bytes = bass.Bass.to_json_bytes
_WAIT_LIMITS = {"Drain": 1, "DMACopy": 1}
_DEF_WAIT_LIMIT = 1


def _to_json_split_waits(self, *a, **kw):
    import json as _json
    data = _json.loads(_orig_to_json_bytes(self, *a, **kw))
    nid = 0
    for f in data.get("functions", []):
        for bb in f.get("blocks", []):
            out = []
            for inst in bb.get("instructions", []):
                si = inst.get("sync_info")
                if isinstance(si, dict):
                    w = si.get("on_wait")
                    if isinstance(w, list):
                        k = _WAIT_LIMITS.get(inst.get("opcode"), _DEF_WAIT_LIMIT)
                        if len(w) > k:
                            extra, keep = w[:-k], w[-k:]
                            for wt in extra:
                                out.append({
                                    "debug": inst.get("debug"),
                                    "engine": inst["engine"],
                                    "ins": [], "outs": [],
                                    "name": f"wsplit-{nid}",
                                    "opcode": "Drain",
                                    "sync_info": {"on_update": [],
                                                  "on_wait": [wt]},
                                })
                                nid += 1
                            si["on_wait"] = keep
                out.append(inst)
            bb["instructions"] = out
    return _json.dumps(data).encode()


bass.Bass.to_json_bytes = _to_json_split_waits

B, T, E = 2, 2048, 1024
H, DH = 16, 64
DKV = 256
DR = 32
HL = 4              # heads per core
NG = 4              # head groups
SCALE = 1.0 / math.sqrt(DH + DR)
TG = 512            # query-group width
KC = 128            # key-chunk width
NTG = T // TG       # 4
NKC = T // KC       # 16
EC = E // 128       # 8  e-chunks
CC = DKV // 128     # 2  latent chunks

F32 = mybir.dt.float32
F32R = mybir.dt.float32r
BF16 = mybir.dt.bfloat16
AF = mybir.ActivationFunctionType
ALU = mybir.AluOpType
AX = mybir.AxisListType

SWAP16 = list(range(16, 32)) + list(range(0, 16))

_CACHE = {}


def _r(ap):
    return ap.bitcast(F32R)


def _build_program():
    nc = bass.Bass()

    xT = nc.declare_dram_parameter("xT", [E, T], BF16, isOutput=False)
    wq = nc.declare_dram_parameter("wq", [E, HL * DH], BF16, isOutput=False)
    wqr = nc.declare_dram_parameter("wqr", [E, HL * DR], BF16, isOutput=False)
    wkr = nc.declare_dram_parameter("wkr", [E, DR], BF16, isOutput=False)
    wkvd = nc.declare_dram_parameter("wkvd", [E, DKV], BF16, isOutput=False)
    wku = nc.declare_dram_parameter("wku", [DKV, HL * DH], BF16, isOutput=False)
    wvu = nc.declare_dram_parameter("wvu", [DKV, HL * DH], BF16, isOutput=False)
    wo = nc.declare_dram_parameter("wo", [HL * DH, E], BF16, isOutput=False)
    cosq = nc.declare_dram_parameter("cosq", [HL * DR, T], BF16, isOutput=False)
    sinq = nc.declare_dram_parameter("sinq", [HL * DR, T], BF16, isOutput=False)
    tri = nc.declare_dram_parameter("tri", [128, 128], BF16, isOutput=False)
    sel = nc.declare_dram_parameter("sel", [66, 128], BF16, isOutput=False)
    lobo = nc.declare_dram_parameter("lobo", [66, 1], F32, isOutput=False)
    out = nc.declare_dram_parameter("out", [T, E], BF16, isOutput=True)

    with TileContext(nc) as tc:
        from contextlib import ExitStack

        with ExitStack() as ctx:
            singles = ctx.enter_context(tc.tile_pool(name="singles", bufs=1))
            pool = ctx.enter_context(tc.tile_pool(name="pool", bufs=2))
            psp = ctx.enter_context(tc.tile_pool(name="psp", bufs=1, space="PSUM"))

            # Each issuing engine owns one ~22.5 GB/s DMA ring, so big
            # loads are chunked round-robin across sync/scalar/gpsimd in
            # consumption order (tg0's x + kv/q weights first).
            QS = [nc.sync, nc.scalar, nc.gpsimd]
            qi = [0]

            def ld(out_ap, in_ap):
                QS[qi[0] % 3].dma_start(out=out_ap, in_=in_ap)
                qi[0] += 1

            xt_sb = singles.tile([128, EC, T], BF16)
            xT_r = xT.rearrange("(c p) t -> p c t", p=128)
            wkvd_sb = singles.tile([128, EC, DKV], BF16)
            wkvd_r = wkvd.rearrange("(c p) f -> p c f", p=128)
            wq_sb = singles.tile([128, EC, HL * DH], BF16)
            wq_r = wq.rearrange("(c p) f -> p c f", p=128)
            wqr_sb = singles.tile([128, EC, HL * DR], BF16)
            wkr_sb = singles.tile([128, EC, DR], BF16)
            for e0 in range(0, EC, 2):
                ld(wkvd_sb[:, e0:e0 + 2, :], wkvd_r[:, e0:e0 + 2, :])
                ld(xt_sb[:, e0:e0 + 2, 0:TG], xT_r[:, e0:e0 + 2, 0:TG])
            nc.sync.dma_start(
                out=wkr_sb, in_=wkr.rearrange("(c p) f -> p c f", p=128))
            for e0 in range(0, EC, 2):
                ld(wq_sb[:, e0:e0 + 2, :], wq_r[:, e0:e0 + 2, :])
            ld(wqr_sb, wqr.rearrange("(c p) f -> p c f", p=128))
            wku_sb = singles.tile([128, CC, HL * DH], BF16)
            ld(wku_sb, wku.rearrange("(c p) f -> p c f", p=128))
            wvu_sb = singles.tile([128, CC, HL * DH], BF16)
            ld(wvu_sb, wvu.rearrange("(c p) f -> p c f", p=128))
            cosq_sb = singles.tile([128, T], BF16)
            sinq_sb = singles.tile([128, T], BF16)
            ld(cosq_sb[:, 0:TG], cosq[:, 0:TG])
            ld(sinq_sb[:, 0:TG], sinq[:, 0:TG])
            tri_sb = singles.tile([128, 128], BF16)
            nc.sync.dma_start(out=tri_sb, in_=tri[:, :])
            sel_sb = singles.tile([66, 128], BF16)
            nc.sync.dma_start(out=sel_sb, in_=sel[:, :])
            lobo_sb = singles.tile([66, 1], F32)
            nc.sync.dma_start(out=lobo_sb, in_=lobo[:, :])
            c_sb = singles.tile([66, 1], F32)
            nc.scalar.activation(c_sb, lobo_sb, AF.Exp)
            for t0 in range(TG, T, TG):
                tsl = slice(t0, t0 + TG)
                ld(cosq_sb[:, tsl], cosq[:, tsl])
                ld(sinq_sb[:, tsl], sinq[:, tsl])
            wo_r = wo.rearrange("(c p) e -> p c e", p=128)
            wo_sb = singles.tile([128, 2, E], BF16)
            ld(wo_sb[:, :, 0:TG], wo_r[:, :, 0:TG])
            ld(wo_sb[:, :, TG:2 * TG], wo_r[:, :, TG:2 * TG])

            # ---------------- persistent activation tiles ----------------
            latT_sb = singles.tile([128, CC, T], BF16)
            # head h lives at slot SLOT[h] so staging DMAs can pair heads
            SLOT = [0, 2, 1, 3]
            qTall = singles.tile([96, HL, T], BF16)
            kTall = singles.tile([96, HL, T], BF16)
            v_sb = singles.tile([128, NKC, HL, DH + 1], BF16)
            nc.vector.memset(v_sb[:, :, :, DH:DH + 1], 1.0)
            yT2 = singles.tile([128, 2, T], BF16)
            # head h row lives at partition 64*(h//2) + h%2
            dsum_sb = singles.tile([66, T], F32)
            emax_sb = singles.tile([66, T], F32)
            emst_sb = singles.tile([66, T], F32)
            nc.vector.memset(dsum_sb, 1.0)
            nc.vector.memset(emax_sb, 1.0)
            nc.vector.memset(emst_sb, 1.0)
            comb2 = singles.tile([128, 2, T], BF16)
            r_bf = singles.tile([66, T], BF16)

            # =================== projections, per tg ===================
            for tg in range(NTG):
                ts = slice(tg * TG, (tg + 1) * TG)
                if tg > 0:
                    ld(xt_sb[:, 0:4, ts], xT_r[:, 0:4, ts])
                    ld(xt_sb[:, 4:8, ts], xT_r[:, 4:8, ts])
                xts = [xt_sb[:, ec, ts] for ec in range(EC)]

                # --- latent (kv) halves + k_rope into one S tile ---
                skv = psp.tile([128, 3 * TG], F32, name="skv", tag="S", bufs=2)
                for ec in range(EC):
                    nc.tensor.matmul(
                        skv[:, 0:TG], wkvd_sb[:, ec, 0:128], xts[ec],
                        start=(ec == 0), stop=(ec == EC - 1))
                for ec in range(EC):
                    nc.tensor.matmul(
                        skv[:, TG:2 * TG], wkvd_sb[:, ec, 128:256], xts[ec],
                        start=(ec == 0), stop=(ec == EC - 1))
                for ec in range(EC):
                    nc.tensor.matmul(
                        skv[0:DR, 2 * TG:3 * TG], wkr_sb[:, ec, :], xts[ec],
                        start=(ec == 0), stop=(ec == EC - 1))
                nc.scalar.copy(
                    latT_sb[:, :, ts],
                    skv[:, 0:2 * TG].rearrange("p (c t) -> p c t", c=2))
                kr_pre = pool.tile([DR, TG], BF16, name="krp", tag="krp", bufs=2)
                nc.scalar.copy(kr_pre, skv[0:DR, 2 * TG:3 * TG])

                # k_rope rotate-half + tables
                kr_sw = pool.tile([DR, TG], BF16, name="krs", tag="krs", bufs=2)
                nc.vector.stream_shuffle(kr_sw, kr_pre, mask=SWAP16)
                kr_m = pool.tile([DR, TG], BF16, name="krm", tag="krm", bufs=2)
                nc.vector.tensor_mul(kr_m, kr_pre, cosq_sb[0:DR, ts])
                nc.vector.tensor_mul(kr_sw, kr_sw, sinq_sb[0:DR, ts])
                nc.vector.tensor_add(kr_m, kr_m, kr_sw)
                for h in range(HL):
                    nc.sync.dma_start(
                        out=kTall[DH:96, SLOT[h], ts], in_=kr_m)

                # --- q halves + q_rope into one S tile ---
                sq = psp.tile([128, 3 * TG], F32, name="sq", tag="S", bufs=2)
                for ec in range(EC):
                    nc.tensor.matmul(
                        sq[:, 0:TG], wq_sb[:, ec, 0:128], xts[ec],
                        start=(ec == 0), stop=(ec == EC - 1))
                for ec in range(EC):
                    nc.tensor.matmul(
                        sq[:, TG:2 * TG], wq_sb[:, ec, 128:256], xts[ec],
                        start=(ec == 0), stop=(ec == EC - 1))
                for ec in range(EC):
                    nc.tensor.matmul(
                        sq[:, 2 * TG:3 * TG], wqr_sb[:, ec, :], xts[ec],
                        start=(ec == 0), stop=(ec == EC - 1))
                stq = pool.tile([128, 2 * TG], BF16, name="stq", tag="stq", bufs=2)
                nc.scalar.copy(stq, sq[:, 0:2 * TG])
                nc.sync.dma_start(
                    out=qTall[0:DH, 0:2, ts],
                    in_=stq[0:DH, :].rearrange("p (k t) -> p k t", k=2))
                nc.sync.dma_start(
                    out=qTall[0:DH, 2:4, ts],
                    in_=stq[DH:128, :].rearrange("p (k t) -> p k t", k=2))
                rp_pre = pool.tile([128, TG], BF16, name="rpp", tag="rpp", bufs=2)
                nc.scalar.copy(rp_pre, sq[:, 2 * TG:3 * TG])
                rp_sw = pool.tile([128, TG], BF16, name="rps", tag="rps", bufs=2)
                nc.vector.stream_shuffle(rp_sw, rp_pre, mask=SWAP16)
                rp_m = pool.tile([128, TG], BF16, name="rpm", tag="rpm", bufs=2)
                nc.vector.tensor_mul(rp_m, rp_pre, cosq_sb[:, ts])
                nc.vector.tensor_mul(rp_sw, rp_sw, sinq_sb[:, ts])
                nc.vector.tensor_add(rp_m, rp_m, rp_sw)
                for h in range(HL):
                    nc.sync.dma_start(
                        out=qTall[DH:96, SLOT[h], ts],
                        in_=rp_m[h * DR:(h + 1) * DR, :])

                # --- k_c from latent ---
                skc = psp.tile([128, 3 * TG], F32, name="skc", tag="S", bufs=2)
                for cc in range(CC):
                    nc.tensor.matmul(
                        skc[:, 0:TG], wku_sb[:, cc, 0:128], latT_sb[:, cc, ts],
                        start=(cc == 0), stop=(cc == CC - 1))
                for cc in range(CC):
                    nc.tensor.matmul(
                        skc[:, TG:2 * TG], wku_sb[:, cc, 128:256],
                        latT_sb[:, cc, ts],
                        start=(cc == 0), stop=(cc == CC - 1))
                stk = pool.tile([128, 2 * TG], BF16, name="stk", tag="stq", bufs=2)
                nc.scalar.copy(stk, skc[:, 0:2 * TG])
                nc.sync.dma_start(
                    out=kTall[0:DH, 0:2, ts],
                    in_=stk[0:DH, :].rearrange("p (k t) -> p k t", k=2))
                nc.sync.dma_start(
                    out=kTall[0:DH, 2:4, ts],
                    in_=stk[DH:128, :].rearrange("p (k t) -> p k t", k=2))

                # --- V (natural layout) for this tg's 4 key chunks ---
                for half in range(2):
                    kc0 = 4 * tg + 2 * half
                    vps = psp.tile([128, TG], F32, name="vps", tag="Y", bufs=2)
                    for cc in range(CC):
                        nc.tensor.matmul(
                            vps[:, 0:256],
                            latT_sb[:, cc, kc0 * KC:(kc0 + 1) * KC],
                            wvu_sb[:, cc, :],
                            start=(cc == 0), stop=(cc == CC - 1))
                    for cc in range(CC):
                        nc.tensor.matmul(
                            vps[:, 256:512],
                            latT_sb[:, cc, (kc0 + 1) * KC:(kc0 + 2) * KC],
                            wvu_sb[:, cc, :],
                            start=(cc == 0), stop=(cc == CC - 1))
                    nc.scalar.copy(
                        v_sb[:, kc0:kc0 + 2, :, 0:DH],
                        vps.rearrange("p (k h d) -> p k h d", k=2, h=HL))

            # =================== attention ===================
            # two heads of a pair run as interleaved independent pipelines
            for hp2 in range(2):
                heads = (2 * hp2, 2 * hp2 + 1)
                for qg in range(NTG):
                    qs = slice(qg * TG, (qg + 1) * TG)
                    nkc = 4 * (qg + 1)
                    groups = [list(range(g0, min(g0 + 3, nkc)))
                              for g0 in range(0, nkc, 3)]
                    yp = {}
                    for h in heads:
                        yp[h] = psp.tile(
                            [128, TG], F32, name=f"yps{h % 2}", tag="Y",
                            bufs=2)
                    pts = {}
                    prev_pv = None

                    def joff(c):
                        # valid columns of chunk c start at j*KC (diag)
                        j = c - (nkc - 4)
                        return j * KC if j > 0 else 0

                    def emit_pv(gi):
                        for h in heads:
                            ptt, chunks = pts[(gi, h)]
                            for li, c in enumerate(chunks):
                                o = joff(c)
                                nc.tensor.matmul(
                                    yp[h][0:DH + 1, o:TG], v_sb[:, c, h, :],
                                    ptt[:, li, o:TG],
                                    start=(c == 0), stop=(c == nkc - 1),
                                    skip_group_check=True)

                    for gi, chunks in enumerate(groups):
                        sp = {}
                        for h in heads:
                            sp[h] = psp.tile(
                                [128, 3 * TG], F32, name=f"sps{h % 2}",
                                tag="S", bufs=2)
                            for li, c in enumerate(chunks):
                                o = joff(c)
                                nc.tensor.matmul(
                                    sp[h][:, li * TG + o:(li + 1) * TG],
                                    kTall[:, SLOT[h], c * KC:(c + 1) * KC],
                                    qTall[:, SLOT[h],
                                          qg * TG + o:(qg + 1) * TG])
                        if prev_pv is not None:
                            emit_pv(prev_pv)
                        for h in heads:
                            ptt = pool.tile(
                                [128, 3, TG], BF16, name=f"pt{h % 2}",
                                tag="pt", bufs=5)
                            pts[(gi, h)] = (ptt, chunks)
                            li = 0
                            while li < len(chunks):
                                c = chunks[li]
                                j = c - (nkc - 4)
                                if j < 0:
                                    l1 = li
                                    while (l1 < len(chunks)
                                           and chunks[l1] - (nkc - 4) < 0):
                                        l1 += 1
                                    nc.scalar.activation(
                                        ptt[:, li:l1, :],
                                        sp[h][:, li * TG:l1 * TG].rearrange(
                                            "p (k t) -> p k t", k=l1 - li),
                                        AF.Exp, scale=SCALE)
                                    li = l1
                                else:
                                    o = j * KC
                                    nc.scalar.activation(
                                        ptt[:, li, o:TG],
                                        sp[h][:, li * TG + o:(li + 1) * TG],
                                        AF.Exp, scale=SCALE)
                                    nc.gpsimd.tensor_mul(
                                        ptt[:, li, o:o + KC],
                                        ptt[:, li, o:o + KC], tri_sb)
                                    li += 1
                        for h in heads:
                            ptt, _ = pts[(gi, h)]
                            cslot = comb2[:, h % 2, :]
                            for li, c in enumerate(chunks):
                                o = joff(c)
                                if c == 0:
                                    nc.vector.tensor_copy(
                                        cslot[:, qs], ptt[:, li, :])
                                else:
                                    nc.vector.tensor_max(
                                        cslot[:, qg * TG + o:(qg + 1) * TG],
                                        cslot[:, qg * TG + o:(qg + 1) * TG],
                                        ptt[:, li, o:TG])
                        prev_pv = gi
                    emit_pv(prev_pv)

                    for h in heads:
                        # stage y + D, scatter to yT2 / dsum
                        hp = 64 * (h // 2) + h % 2
                        st65 = pool.tile(
                            [DH + 1, TG], F32, name="st65", tag="st65",
                            bufs=4)
                        nc.vector.tensor_copy(st65, yp[h][0:DH + 1, :])
                        nc.gpsimd.dma_start(
                            out=yT2[(h % 2) * DH:(h % 2 + 1) * DH,
                                    h // 2, qs],
                            in_=st65[0:DH, :])
                        nc.sync.dma_start(
                            out=dsum_sb[hp:hp + 1, qs],
                            in_=st65[DH:DH + 1, :])

                # per-head partition-max via 32x32 transpose + 2 reduces
                for h in heads:
                    hp = 64 * (h // 2) + h % 2
                    combT = pool.tile(
                        [128, T], BF16, name="combT", tag="combT", bufs=1)
                    nc.vector.transpose(combT, comb2[:, h % 2, :])
                    red = pool.tile(
                        [128, T // 32], F32, name="red", tag="red", bufs=2)
                    nc.vector.reduce_max(
                        red, combT.rearrange("p (b j) -> p b j", j=32),
                        axis=AX.X)
                    stkt = pool.tile(
                        [32, 4, T // 32], F32, name="stkt", tag="stkt",
                        bufs=2)
                    for a in range(4):
                        nc.sync.dma_start(
                            out=stkt[:, a, :],
                            in_=red[a * 32:(a + 1) * 32, :])
                    emf = pool.tile(
                        [32, T // 32], F32, name="emf", tag="emf", bufs=2)
                    nc.vector.reduce_max(
                        emf, stkt.rearrange("p a b -> p b a"), axis=AX.X)
                    nc.sync.dma_start(
                        out=emst_sb[hp:hp + 1, :].rearrange(
                            "p (i b) -> p i b", i=32),
                        in_=emf)

                # this pair's denominator chain (rows 64*hp2 .. +2) can run
                # during the next pair's attention / the out projection
                rows = slice(64 * hp2, 64 * hp2 + 2)
                for tg in range(NTG):
                    ts = slice(tg * TG, (tg + 1) * TG)
                    nc.vector.tensor_copy(
                        emax_sb[rows, ts].rearrange("p (b i) -> p i b", i=32),
                        emst_sb[rows, :].rearrange(
                            "p (i b) -> p i b", b=64)[:, :, 16 * tg:16 * (tg + 1)])
                    nc.vector.scalar_tensor_tensor(
                        out=dsum_sb[rows, ts], in0=emax_sb[rows, ts],
                        scalar=c_sb[rows, :], in1=dsum_sb[rows, ts],
                        op0=ALU.mult, op1=ALU.add)
                    lnd = pool.tile(
                        [2, TG], F32, name="lnd", tag="lnd", bufs=2)
                    nc.scalar.activation(lnd, dsum_sb[rows, ts], AF.Ln)
                    nc.scalar.activation(
                        r_bf[rows, ts], lnd, AF.Exp, scale=-1.0)

            # ========== normalize + out-proj, per tg ==========
            for tg in range(NTG):
                ts = slice(tg * TG, (tg + 1) * TG)
                for g in range(2):
                    bcps = psp.tile([128, TG], F32, name="bc", tag="Y", bufs=2)
                    nc.tensor.matmul(
                        bcps, sel_sb[64 * g:64 * g + 2, :],
                        r_bf[64 * g:64 * g + 2, ts])
                    nc.vector.tensor_mul(
                        yT2[:, g, ts], yT2[:, g, ts], bcps)
                for tt in range(4 * tg, 4 * (tg + 1)):
                    for eg in range(2):
                        ops = psp.tile(
                            [128, TG], F32, name="ops", tag="Y", bufs=2)
                        for fc in range(2):
                            nc.tensor.matmul(
                                ops, yT2[:, fc, tt * KC:(tt + 1) * KC],
                                wo_sb[:, fc, eg * TG:(eg + 1) * TG],
                                start=(fc == 0), stop=(fc == 1))
                        ost = pool.tile(
                            [128, TG], BF16, name="ost", tag="ost", bufs=3)
                        if (tt * 2 + eg) % 2 == 0:
                            nc.scalar.copy(ost, ops)
                        else:
                            nc.vector.tensor_copy(ost, ops)
                        nc.sync.dma_start(
                            out=out[tt * KC:(tt + 1) * KC,
                                    eg * TG:(eg + 1) * TG],
                            in_=ost)

    return nc


def _tri():
    x = np.arange(128)[:, None]
    y = np.arange(128)[None, :]
    return (x <= y).astype(np.float32)  # keep iff key row <= query col


def _sel():
    s = np.zeros((66, 128), dtype=np.float32)
    for base in (0, 64):
        s[base + 0, 0:64] = 1.0
        s[base + 1, 64:128] = 1.0
    return s


def _lobo66(lg):
    v = np.zeros((66, 1), dtype=np.float32)
    for h in range(HL):
        v[64 * (h // 2) + h % 2, 0] = lg[h]
    return v


def _rope_tables():
    half = DR // 2
    inv = 1.0 / (10000.0 ** (np.arange(half, dtype=np.float64) / half))
    ang = np.arange(T, dtype=np.float64)[:, None] * inv[None, :]  # (T, half)
    cos = np.cos(ang).T  # (half, T)
    sin = np.sin(ang).T
    cosk = np.concatenate([cos, cos], axis=0)                 # (32, T)
    sink = np.concatenate([-sin, sin], axis=0)
    cosq = np.tile(cosk, (HL, 1)).astype(np.float32)          # (128, T)
    sinq = np.tile(sink, (HL, 1)).astype(np.float32)
    return cosq, sinq


def kernel(x, Wq, Wqr, Wkr, Wkvd, Wku, Wvu, Wo, lobo_log):
    x = np.asarray(x, dtype=np.float32)
    Wq = np.asarray(Wq, dtype=np.float32)
    Wqr = np.asarray(Wqr, dtype=np.float32)
    Wkr = np.asarray(Wkr, dtype=np.float32)
    Wkvd = np.asarray(Wkvd, dtype=np.float32)
    Wku = np.asarray(Wku, dtype=np.float32)
    Wvu = np.asarray(Wvu, dtype=np.float32)
    Wo = np.asarray(Wo, dtype=np.float32)
    lobo_log = np.asarray(lobo_log, dtype=np.float32)

    if "nc" not in _CACHE:
        _CACHE["nc"] = _build_program()
    nc = _CACHE["nc"]

    cosq, sinq = _rope_tables()
    bf = lambda a: np.ascontiguousarray(a).astype(BF16NP)
    xTb = [bf(x[b].T) for b in range(B)]
    wkr_b, wkvd_b = bf(Wkr), bf(Wkvd)
    cosq_b, sinq_b = bf(cosq), bf(sinq)
    tri_b, sel_b = bf(_tri()), bf(_sel())
    in_maps = []
    for core in range(8):
        b, g = core // NG, core % NG
        hs = slice(g * HL * DH, (g + 1) * HL * DH)
        rs = slice(g * HL * DR, (g + 1) * HL * DR)
        in_maps.append({
            "xT": xTb[b],
            "wq": bf(Wq[:, hs]),
            "wqr": bf(Wqr[:, rs]),
            "wkr": wkr_b,
            "wkvd": wkvd_b,
            "wku": bf(Wku[:, hs]),
            "wvu": bf(Wvu[:, hs]),
            "wo": bf(Wo[hs, :]),
            "cosq": cosq_b, "sinq": sinq_b,
            "tri": tri_b, "sel": sel_b,
            "lobo": _lobo66(lobo_log[g * HL:(g + 1) * HL]),
        })

    trace = bool(os.environ.get("BASS_TRACE_KERNEL"))
    bkr = run_bass_kernel_spmd(
        nc, in_maps, core_ids=list(range(8)), trace=trace)
    if trace:
        print(f"HW exec time: {bkr.exec_time_ns} ns")
        if bkr.instructions_and_trace is not None:
            print("trace:", bkr.instructions_and_trace[1])
        _CACHE["last_result"] = bkr
    res = bkr.results
    out = np.zeros((B, T, E), dtype=np.float32)
    for core in range(8):
        out[core // NG] += np.asarray(res[core]["out"], dtype=np.float32)
    return out
